# revision 1
# baseline (speedup 1.0000x reference)
"""Trainium2 Bass kernel for nn_DihedralAngleLayer.

Input:  x [2_000_000, 42] f32 (14 atoms x 3 coords per row),
        mask_matrix [4, 14] f32 one-hot carbon selector.
Output: dihedral angle per row, [2_000_000] f32.

Data-parallel across 8 NeuronCores: rows are padded to 8*250_112 and split
evenly. Each core owns rows in global partition-major order: partition p
handles rows [p*Q, (p+1)*Q), Q = rows/128. Per tile (G columns of every
partition) the Vector engine computes

    a = c0-c1, b = c2-c1, d = c3-c2, m = b x d
    r=a.b p=a.d det=a.m q=b.b s=b.d       (dup-write + shifted-AP crosses)
    xx = p*q - r*s        # Lagrange identity for (a x b).(d x b)
    yy = sqrt(q)*det      # |v1| * scalar triple product

writing xx,yy into full-length planes; the per-row-scalar atan2 tail
(range-reduced arctan on ScalarE) runs on multi-tile chunks so the ~0.5us
per-instruction floor amortizes. dm2/da1 run on GPSIMD to offload DVE.
"""

import numpy as np

import concourse.bacc as bacc
import concourse.bass as bass
import concourse.mybir as mybir
from concourse.bass_utils import run_bass_kernel_spmd
from concourse.tile import TileContext

AF = mybir.ActivationFunctionType
OP = mybir.AluOpType
F32 = mybir.dt.float32

PI = float(np.pi)

N_CORES = 8
G_TILE = 192
# first two tiles halved so DVE starts ~4us sooner (smaller first DMA);
# trailing 34-tile gets its own chunk so almost no tail work is exposed
# after the last head finishes.
TILES = [96, 96] + [G_TILE] * 9 + [34]   # sum = 1954
CHUNK_AFTER = {3, 7, 10, 11}             # tile indices closing a tail chunk
Q = sum(TILES)                      # rows per partition
ROWS_PER_CORE = 128 * Q            # 250_112
TILES_PER_CHUNK = 4

# row-interleaved scratch layout per row-group (period 39 floats)
PER = 39
S_A, S_B2, S_D2, S_M = 0, 3, 9, 15
P_1, P_2 = 18, 21
D_0 = 24
# per-tile mini-planes ([G] each) for dots + pq/rs/sq: r,p,det,q,s,pq,rs,sq
RP_R, RP_P, RP_DET, RP_Q, RP_S, RP_PQ, RP_RS, RP_SQ = range(8)

# chunk-tail scratch: 8 slots of CS_FD floats each (slots reused over the chain)
CS_FD = G_TILE * TILES_PER_CHUNK


def _ap(base, off, dims):
    return bass.AP(
        base.tensor, base.offset + off, [list(base.ap[0])] + [list(d) for d in dims]
    )


def _emit_head(nc, xp, scp, rp, x, xyf, toff, G, c0, c1, c2, c3):
    """Per-tile head: subs, cross, dots, xx/yy -> full-length planes."""
    v, s, g = nc.vector, nc.scalar, nc.gpsimd

    xt = xp.tile([128, G * 42], F32, tag="x")
    sc = scp.tile([128, G * PER], F32, tag="sc")
    r5 = rp.tile([128, G * 8], F32, tag="r5")

    nc.gpsimd.dma_start(
        out=xt[:],
        in_=x.rearrange("(p q) c -> p q c", p=128)[:, toff : toff + G, :],
    )

    xa, sa, ra = xt[:], sc[:], r5[:]

    def xap(off, dims):
        return _ap(xa, off, [[42, G]] + dims)

    def sap(off, dims=()):
        return _ap(sa, off, [[PER, G]] + list(dims))

    def rap(k, dims=None):
        return _ap(ra, k * G, dims if dims is not None else [[1, G]])

    # a = c0-c1
    v.tensor_tensor(sap(S_A, [[1, 3]]), xap(c0, [[1, 3]]), xap(c1, [[1, 3]]), OP.subtract)
    # duplicated b = c2-c1 and d = c3-c2 (ISA allows max 3 free dims per AP)
    v.tensor_tensor(
        sap(S_B2, [[3, 2], [1, 3]]),
        xap(c2, [[0, 2], [1, 3]]),
        xap(c1, [[0, 2], [1, 3]]),
        OP.subtract,
    )
    v.tensor_tensor(
        sap(S_D2, [[3, 2], [1, 3]]),
        xap(c3, [[0, 2], [1, 3]]),
        xap(c2, [[0, 2], [1, 3]]),
        OP.subtract,
    )
    # P1 = b_yzx*d_zxy ; P2 = b_zxy*d_yzx
    v.tensor_tensor(
        sap(P_1, [[3, 2], [1, 3]]),
        sap(S_B2 + 1, [[1, 2], [1, 3]]),
        sap(S_D2 + 2, [[-1, 2], [1, 3]]),
        OP.mult,
    )
    # m = P1 - P2
    v.tensor_tensor(sap(S_M, [[1, 3]]), sap(P_1, [[1, 3]]), sap(P_2, [[1, 3]]), OP.subtract)
    # three-prods of a with {b,d,m} -> rt,pt,dett   (DVE)
    v.tensor_tensor(
        sap(D_0, [[3, 3], [1, 3]]),
        sap(S_A, [[0, 3], [1, 3]]),
        sap(S_B2, [[6, 3], [1, 3]]),
        OP.mult,
    )
    # qt = b*b on ScalarE (Square is a filler in every ACT table set, and the
    # Scalar engine is far from saturated); st = b*d stays on DVE.
    # (GPSIMD tensor work is a net loss: it contends with DVE on the shared
    #  SBUF port and inflates every concurrent DVE op up to 2.4x — measured.)
    s.activation(sap(D_0 + 9, [[1, 3]]), sap(S_B2, [[1, 3]]), AF.Square)
    v.tensor_tensor(
        sap(D_0 + 12, [[1, 3]]),
        sap(S_B2, [[1, 3]]),
        sap(S_D2, [[1, 3]]),
        OP.mult,
    )
    # segmented reduce -> dots r,p,det,q,s as per-tile mini-planes: iterating
    # (dot, row, comp) makes both the reads and the plane writes unit-inner
    v.reduce_sum(
        rap(RP_R, [[G, 5], [1, G]]),
        _ap(sa, D_0, [[3, 5], [PER, G], [1, 3]]),
        axis=mybir.AxisListType.X,
    )
    # [pq, rs] on planes
    v.tensor_tensor(
        rap(RP_PQ, [[G, 2], [1, G]]),
        rap(RP_P, [[-G, 2], [1, G]]),
        rap(RP_Q, [[G, 2], [1, G]]),
        OP.mult,
    )
    # xx -> full plane (all unit stride)
    v.tensor_tensor(_ap(xyf, toff, [[1, G]]), rap(RP_PQ), rap(RP_RS), OP.subtract)
    # sq = sqrt(q); yy = sq*det -> full plane
    s.activation(rap(RP_SQ), rap(RP_Q), AF.Sqrt)
    v.tensor_tensor(_ap(xyf, Q + toff, [[1, G]]), rap(RP_SQ), rap(RP_DET), OP.mult)


def _emit_tail(nc, csp, outp, y, xyf, toff, FD):
    """Chunk tail: atan2 on [128, FD] contiguous planes."""
    v, s = nc.vector, nc.scalar

    cs = csp.tile([128, 7 * CS_FD], F32, tag="cs")
    ot = outp.tile([128, CS_FD], F32, tag="o")
    ca = cs[:]

    def cap(k, n=1):
        return _ap(ca, k * CS_FD, [[1, FD]] if n == 1 else [[CS_FD, n], [1, FD]])

    def xy(n=1):
        return _ap(xyf, toff, [[Q, n], [1, FD]] if n > 1 else [[1, FD]])

    # slots: 0:ax/e2  1:ay/sy  2:df/u  3:mn/v  4:mx/rq  5:rmx/al  6:e1/z
    s.activation(cap(0, 2), xy(2), AF.Abs)                       # ax,ay
    v.tensor_tensor(cap(2), cap(0), cap(1), OP.subtract)          # df
    v.tensor_tensor(cap(3), cap(0), cap(1), OP.min)               # mn
    v.tensor_tensor(cap(4), cap(0), cap(1), OP.max)               # mx
    v.reciprocal_approx_fast(cap(5), cap(4))                      # rmx
    v.tensor_tensor(cap(4), cap(3), cap(5), OP.mult)              # rq (mx slot)
    s.activation(cap(5), cap(4), AF.Arctan)                       # al (rmx slot)
    s.activation(cap(6), cap(2), AF.Sign)                         # e1
    s.activation(cap(0, 2), xy(2), AF.Sign)                       # e2,sy (ax/ay slots)
    v.tensor_tensor(cap(2), cap(6), cap(0), OP.mult)              # u (df slot)
    v.tensor_tensor(cap(3), cap(5), cap(2), OP.mult)              # v (mn slot)
    v.scalar_tensor_tensor(cap(5), cap(2), PI / 4, cap(3), OP.mult, OP.subtract)  # w2
    v.scalar_tensor_tensor(cap(6), cap(0), PI / 4, cap(5), OP.mult, OP.add)       # z
    v.scalar_tensor_tensor(
        _ap(ot[:], 0, [[1, FD]]), cap(6), PI / 2, cap(1), OP.subtract, OP.mult
    )
    nc.gpsimd.dma_start(
        out=y.rearrange("(p q) -> p q", p=128)[:, toff : toff + FD],
        in_=_ap(ot[:], 0, [[1, FD]]),
    )


def build_kernel(atoms):
    c0, c1, c2, c3 = (3 * int(a) for a in atoms)
    nc = bacc.Bacc("TRN2", target_bir_lowering=False, debug=False)
    x = nc.dram_tensor("x", [ROWS_PER_CORE, 42], F32, kind="ExternalInput")
    y = nc.dram_tensor("y", [ROWS_PER_CORE], F32, kind="ExternalOutput")
    with TileContext(nc) as tc:
        with (
            tc.tile_pool(name="xp", bufs=2) as xp,
            tc.tile_pool(name="scp", bufs=2) as scp,
            tc.tile_pool(name="rp", bufs=2) as rp,
            tc.tile_pool(name="xyp", bufs=1) as xyp,
            tc.tile_pool(name="csp", bufs=1) as csp,
            tc.tile_pool(name="outp", bufs=2) as outp,
        ):
            xyf_tile = xyp.tile([128, 2 * Q], F32, tag="xy")
            xyf = xyf_tile[:]
            toff = 0
            chunk_start = 0
            for i, G in enumerate(TILES):
                _emit_head(nc, xp, scp, rp, x, xyf, toff, G, c0, c1, c2, c3)
                toff += G
                if i in CHUNK_AFTER or i == len(TILES) - 1:
                    _emit_tail(nc, csp, outp, y, xyf, chunk_start, toff - chunk_start)
                    chunk_start = toff
    nc.finalize()
    return nc


_CACHE = {}


def _get_nc(atoms):
    key = tuple(int(a) for a in atoms)
    if key not in _CACHE:
        _CACHE[key] = build_kernel(key)
    return _CACHE[key]


def run(x, atoms=(0, 4, 7, 11), **spmd_kwargs):
    """x: [B, 42] f32. Returns (y [B] f32, BassKernelResults)."""
    x = np.ascontiguousarray(np.asarray(x, dtype=np.float32))
    B = x.shape[0]
    total = N_CORES * ROWS_PER_CORE
    if B < total:
        # pad with replicated leading rows (valid, non-degenerate data)
        x = np.concatenate([x, x[: total - B]], axis=0)
    nc = _get_nc(atoms)
    shards = x.reshape(N_CORES, ROWS_PER_CORE, 42)
    in_maps = [{"x": shards[i]} for i in range(N_CORES)]
    res = run_bass_kernel_spmd(nc, in_maps, core_ids=list(range(N_CORES)), **spmd_kwargs)
    y = np.concatenate([r["y"] for r in res.results])[:B]
    return np.asarray(y, dtype=np.float32), res


def kernel(x, mask_matrix):
    mask = np.asarray(mask_matrix)
    atoms = tuple(int(i) for i in np.argmax(mask, axis=1))
    y, _ = run(x, atoms=atoms)
    return y



# revision 3
# speedup vs baseline: 1.1536x; 1.1536x over previous
"""Trainium2 Bass kernel for nn_DihedralAngleLayer.

Input:  x [2_000_000, 42] f32 (14 atoms x 3 coords per row),
        mask_matrix [4, 14] f32 one-hot carbon selector.
Output: dihedral angle per row, [2_000_000] f32.

Data-parallel across 8 NeuronCores: rows are padded to 8*250_112 and split
evenly. Each core owns rows in global partition-major order: partition p
handles rows [p*Q, (p+1)*Q), Q = rows/128.

Per tile (G rows of every partition) the Vector engine computes

    a = c0-c1, b = c2-c1, d = c3-c2, m = d x b  (3 split-AP multiplies, no
    duplicated subs), prods a*(b|d|m) and b*d, dots via two 5-plane strided
    adds (cheaper than the 1x-mode reduce), then
    xx = p*q - r*s, t2 = sqrt(q)*det / xx  (reciprocal_approx_fast).

The tail is a signed-ratio arctan: out = arctan(t2) + sign(t2)*(-pi*[xx<0]),
so per row only arctan + 2 tensor ops remain.  ScalarE ops are split so
heads touch only sqrt_and_others table funcs (Square/Sqrt) and tails only
sigmoid_and_others funcs (Arctan/Sign/Copy), bounding ACT table reloads to
two per chunk.  DVE model: (N+151)/0.96 ns per op -> ~46 elems/row + 15
instrs/tile; big G amortizes the per-instruction overhead.
"""

import numpy as np

import concourse.bacc as bacc
import concourse.bass as bass
import concourse.mybir as mybir
from concourse.bass_utils import run_bass_kernel_spmd
from concourse.tile import TileContext

AF = mybir.ActivationFunctionType
OP = mybir.AluOpType
F32 = mybir.dt.float32

PI = float(np.pi)

N_CORES = 8
# 7 tiles: two small leading tiles start DVE sooner; sum = 1954
TILES = [160, 166, 326, 326, 326, 326, 324]
CHUNK_AFTER = {2, 4}                 # tail chunks close after these + last
Q = sum(TILES)                       # rows per partition (1954)
ROWS_PER_CORE = 128 * Q              # 250_112
CHUNK_MAX = 652

# per-row scratch layout (period PER floats)
PER = 33
S_A, S_B, S_D, S_M, S_P1, S_P2, S_PR = 0, 3, 6, 9, 12, 15, 18
# prods: r@18 p@21 det@24 s@27 q@30 (3 each)
# mini planes ([G] each): r p det s q pq rs rxx sq yy
M_R, M_P, M_DET, M_S, M_Q, M_PQ, M_RS, M_RXX, M_SQ, M_YY = range(10)


def _ap(base, off, dims):
    return bass.AP(
        base.tensor, base.offset + off, [list(base.ap[0])] + [list(d) for d in dims]
    )


def _emit_head(nc, xp, scp, mp, x, planes, toff, G, c0, c1, c2, c3):
    """Per-tile head: subs, cross, dots, xx/t2 -> full-length planes."""
    v, s = nc.vector, nc.scalar

    xt = xp.tile([128, G * 42], F32, tag="x")
    sc = scp.tile([128, G * PER], F32, tag="sc")
    mi = mp.tile([128, G * 10], F32, tag="mi")

    nc.gpsimd.dma_start(
        out=xt[:],
        in_=x.rearrange("(p q) c -> p q c", p=128)[:, toff : toff + G, :],
    )

    xa, sa, ma = xt[:], sc[:], mi[:]

    def xap(off, dims):
        return _ap(xa, off, [[42, G]] + dims)

    def sap(off, dims=()):
        return _ap(sa, off, [[PER, G]] + list(dims))

    def map_(k, dims=None):
        return _ap(ma, k * G, dims if dims is not None else [[1, G]])

    # a = c0-c1 and b = c2-c1 fused (in0 strides over {c0,c2}, in1 reads c1 twice)
    v.tensor_tensor(
        sap(S_A, [[3, 2], [1, 3]]),
        xap(c0, [[c2 - c0, 2], [1, 3]]),
        xap(c1, [[0, 2], [1, 3]]),
        OP.subtract,
    )
    # d = c3-c2
    v.tensor_tensor(sap(S_D, [[1, 3]]), xap(c3, [[1, 3]]), xap(c2, [[1, 3]]), OP.subtract)
    # q prods = b*b on ScalarE (Square; sqrt_and_others set), overlaps DVE below
    s.activation(sap(S_PR + 12, [[1, 3]]), sap(S_B, [[1, 3]]), AF.Square)
    # cross products, no duplicated storage: 3 split-AP multiplies
    # P1 = (by*dz, bz*dx, bx*dy), P2 = (bz*dy, bx*dz, by*dx)
    v.tensor_tensor(sap(S_P1, [[1, 2]]), sap(S_B + 1, [[1, 2]]), sap(S_D + 2, [[-2, 2]]), OP.mult)
    v.tensor_tensor(sap(S_P2, [[1, 2]]), sap(S_B + 2, [[-2, 2]]), sap(S_D + 1, [[1, 2]]), OP.mult)
    v.tensor_tensor(sap(S_P1 + 2, [[3, 2]]), sap(S_B, [[1, 2]]), sap(S_D + 1, [[-1, 2]]), OP.mult)
    # m = P2 - P1 = d x b  (orientation chosen so yy = +sqrt(q)*det)
    v.tensor_tensor(sap(S_M, [[1, 3]]), sap(S_P2, [[1, 3]]), sap(S_P1, [[1, 3]]), OP.subtract)
    # prods a*(b|d|m) -> r,p,det ; b*d -> s
    v.tensor_tensor(
        sap(S_PR, [[3, 3], [1, 3]]),
        sap(S_A, [[0, 3], [1, 3]]),
        sap(S_B, [[3, 3], [1, 3]]),
        OP.mult,
    )
    v.tensor_tensor(sap(S_PR + 9, [[1, 3]]), sap(S_B, [[1, 3]]), sap(S_D, [[1, 3]]), OP.mult)
    # dots via two strided adds (10G elems vs 15G for the 1x reduce);
    # iteration (dot k outer, row g inner) on both sides
    v.tensor_tensor(
        map_(M_R, [[G, 5], [1, G]]),
        _ap(sa, S_PR, [[3, 5], [PER, G]]),
        _ap(sa, S_PR + 1, [[3, 5], [PER, G]]),
        OP.add,
    )
    v.tensor_tensor(
        map_(M_R, [[G, 5], [1, G]]),
        map_(M_R, [[G, 5], [1, G]]),
        _ap(sa, S_PR + 2, [[3, 5], [PER, G]]),
        OP.add,
    )
    # pq, rs
    v.tensor_tensor(
        map_(M_PQ, [[G, 2], [1, G]]),
        map_(M_P, [[-G, 2], [1, G]]),
        map_(M_Q, [[-G, 2], [1, G]]),
        OP.mult,
    )
    # xx -> full plane (kept for the tail's sign-of-xx correction)
    v.tensor_tensor(_ap(planes, toff, [[1, G]]), map_(M_PQ), map_(M_RS), OP.subtract)
    # rxx ~ 1/xx (51-ULP custom DVE op), sq = sqrt(q) on ScalarE
    v.reciprocal_approx_fast(map_(M_RXX), _ap(planes, toff, [[1, G]]))
    s.activation(map_(M_SQ), map_(M_Q), AF.Sqrt)
    # t2 = sq*det*rxx -> full plane
    v.tensor_tensor(map_(M_YY), map_(M_SQ), map_(M_DET), OP.mult)
    v.tensor_tensor(_ap(planes, Q + toff, [[1, G]]), map_(M_YY), map_(M_RXX), OP.mult)


def _emit_tail(nc, tsp, outp, y, planes, toff, FD):
    """Chunk tail: out = arctan(t2) + sign(t2) * (-pi * [xx<0])."""
    v, s = nc.vector, nc.scalar

    ts = tsp.tile([128, 3 * CHUNK_MAX], F32, tag="ts")
    ot = outp.tile([128, CHUNK_MAX], F32, tag="o")
    ta = ts[:]

    def cap(k):
        return _ap(ta, k * CHUNK_MAX, [[1, FD]])

    def t2f():
        return _ap(planes, Q + toff, [[1, FD]])

    def xxf():
        return _ap(planes, toff, [[1, FD]])

    # all ScalarE funcs here live in sigmoid_and_others: one table set per tail
    s.activation(cap(0), t2f(), AF.Arctan)                        # al2
    s.activation(cap(1), t2f(), AF.Sign)                          # s2
    s.activation(cap(2), xxf(), AF.Sign, scale=-1.0)              # sign(-xx)
    s.activation(cap(2), cap(2), AF.Copy, scale=-PI / 2, bias=-PI / 2)  # -pi*[xx<0]
    v.tensor_tensor(cap(2), cap(1), cap(2), OP.mult)              # c = s2*cpl
    v.tensor_tensor(_ap(ot[:], 0, [[1, FD]]), cap(0), cap(2), OP.add)
    nc.gpsimd.dma_start(
        out=y.rearrange("(p q) -> p q", p=128)[:, toff : toff + FD],
        in_=_ap(ot[:], 0, [[1, FD]]),
    )


def build_kernel(atoms):
    c0, c1, c2, c3 = (3 * int(a) for a in atoms)
    nc = bacc.Bacc("TRN2", target_bir_lowering=False, debug=False)
    x = nc.dram_tensor("x", [ROWS_PER_CORE, 42], F32, kind="ExternalInput")
    y = nc.dram_tensor("y", [ROWS_PER_CORE], F32, kind="ExternalOutput")
    with TileContext(nc) as tc:
        with (
            tc.tile_pool(name="xp", bufs=2) as xp,
            tc.tile_pool(name="scp", bufs=1) as scp,
            tc.tile_pool(name="mp", bufs=1) as mp,
            tc.tile_pool(name="plp", bufs=1) as plp,
            tc.tile_pool(name="tsp", bufs=1) as tsp,
            tc.tile_pool(name="outp", bufs=2) as outp,
        ):
            pl_tile = plp.tile([128, 2 * Q], F32, tag="pl")
            planes = pl_tile[:]
            toff = 0
            chunk_start = 0
            for i, G in enumerate(TILES):
                _emit_head(nc, xp, scp, mp, x, planes, toff, G, c0, c1, c2, c3)
                toff += G
                if i in CHUNK_AFTER or i == len(TILES) - 1:
                    _emit_tail(nc, tsp, outp, y, planes, chunk_start, toff - chunk_start)
                    chunk_start = toff
    nc.finalize()
    return nc


_CACHE = {}


def _get_nc(atoms):
    key = tuple(int(a) for a in atoms)
    if key not in _CACHE:
        _CACHE[key] = build_kernel(key)
    return _CACHE[key]


def run(x, atoms=(0, 4, 7, 11), **spmd_kwargs):
    """x: [B, 42] f32. Returns (y [B] f32, BassKernelResults)."""
    x = np.ascontiguousarray(np.asarray(x, dtype=np.float32))
    B = x.shape[0]
    total = N_CORES * ROWS_PER_CORE
    if B < total:
        # pad with replicated leading rows (valid, non-degenerate data)
        x = np.concatenate([x, x[: total - B]], axis=0)
    nc = _get_nc(atoms)
    shards = x.reshape(N_CORES, ROWS_PER_CORE, 42)
    in_maps = [{"x": shards[i]} for i in range(N_CORES)]
    res = run_bass_kernel_spmd(nc, in_maps, core_ids=list(range(N_CORES)), **spmd_kwargs)
    y = np.concatenate([r["y"] for r in res.results])[:B]
    return np.asarray(y, dtype=np.float32), res


def kernel(x, mask_matrix):
    mask = np.asarray(mask_matrix)
    atoms = tuple(int(i) for i in np.argmax(mask, axis=1))
    y, _ = run(x, atoms=atoms)
    return y


# revision 7
# speedup vs baseline: 1.2361x; 1.0715x over previous
"""Trainium2 Bass kernel for nn_DihedralAngleLayer.

Input:  x [2_000_000, 42] f32 (14 atoms x 3 coords per row),
        mask_matrix [4, 14] f32 one-hot carbon selector.
Output: dihedral angle per row, [2_000_000] f32.

Data-parallel across 8 NeuronCores: rows are padded to 8*250_112 and split
evenly. Each core owns rows in global partition-major order: partition p
handles rows [p*Q, (p+1)*Q), Q = rows/128.

Per tile (G rows of every partition) the Vector engine computes

    a = c0-c1, b = c2-c1, d = c3-c2, m = d x b  (3 split-AP multiplies, no
    duplicated subs), prods a*(b|d|m) and b*d, dots via two 5-plane strided
    adds (cheaper than the 1x-mode reduce), then
    xx = p*q - r*s, t2 = sqrt(q)*det / xx  (reciprocal_approx_fast).

The tail is a signed-ratio arctan: out = arctan(t2) + sign(t2)*(-pi*[xx<0]),
so per row only arctan + 2 tensor ops remain.  ScalarE ops are split so
heads touch only sqrt_and_others table funcs (Square/Sqrt) and tails only
sigmoid_and_others funcs (Arctan/Sign/Copy), bounding ACT table reloads to
two per chunk.  DVE model: (N+151)/0.96 ns per op -> ~46 elems/row + 15
instrs/tile; big G amortizes the per-instruction overhead.
"""

import numpy as np

import concourse.bacc as bacc
import concourse.bass as bass
import concourse.mybir as mybir
from concourse.bass_utils import run_bass_kernel_spmd
from concourse.tile import TileContext

AF = mybir.ActivationFunctionType
OP = mybir.AluOpType
F32 = mybir.dt.float32

PI = float(np.pi)

N_CORES = 8
# two small leading tiles start DVE sooner; tiny last tile bounds end drain
TILES = [96, 160, 326, 326, 326, 326, 326, 68]
CHUNK_AFTER = {2, 4, 6}              # tail chunks close after these + last
Q = sum(TILES)                       # rows per partition (1954)
ROWS_PER_CORE = 128 * Q              # 250_112
CHUNK_MAX = 652

# per-row scratch layout (period PER floats)
PER = 33
S_A, S_B, S_D, S_M, S_P1, S_P2, S_PR = 0, 3, 6, 9, 12, 15, 18
# prods: r@18 p@21 det@24 s@27 q@30 (3 each)
# mini planes ([G] each): r p det s q pq rs rxx sq yy
M_R, M_P, M_DET, M_S, M_Q, M_PQ, M_RS, M_RXX, M_SQ, M_YY = range(10)


def _ap(base, off, dims):
    return bass.AP(
        base.tensor, base.offset + off, [list(base.ap[0])] + [list(d) for d in dims]
    )


def _emit_head(nc, xp, scp, mp, x, planes, toff, G, c0, c1, c2, c3):
    """Per-tile head: subs, cross, dots, xx/t2 -> full-length planes."""
    v, s = nc.vector, nc.scalar

    xt = xp.tile([128, G * 42], F32, tag="x")
    sc = scp.tile([128, G * PER], F32, tag="sc")
    mi = mp.tile([128, G * 10], F32, tag="mi")

    nc.gpsimd.dma_start(
        out=xt[:],
        in_=x.rearrange("(p q) c -> p q c", p=128)[:, toff : toff + G, :],
    )

    xa, sa, ma = xt[:], sc[:], mi[:]

    def xap(off, dims):
        return _ap(xa, off, [[42, G]] + dims)

    def sap(off, dims=()):
        return _ap(sa, off, [[PER, G]] + list(dims))

    def map_(k, dims=None):
        return _ap(ma, k * G, dims if dims is not None else [[1, G]])

    # a = c0-c1 and b = c2-c1 fused (in0 strides over {c0,c2}, in1 reads c1 twice)
    v.tensor_tensor(
        sap(S_A, [[3, 2], [1, 3]]),
        xap(c0, [[c2 - c0, 2], [1, 3]]),
        xap(c1, [[0, 2], [1, 3]]),
        OP.subtract,
    )
    # d = c3-c2
    v.tensor_tensor(sap(S_D, [[1, 3]]), xap(c3, [[1, 3]]), xap(c2, [[1, 3]]), OP.subtract)
    # q prods = b*b on ScalarE (Square; sqrt_and_others set), overlaps DVE below
    s.activation(sap(S_PR + 12, [[1, 3]]), sap(S_B, [[1, 3]]), AF.Square)
    # cross products, no duplicated storage: 3 split-AP multiplies
    # P1 = (by*dz, bz*dx, bx*dy), P2 = (bz*dy, bx*dz, by*dx)
    v.tensor_tensor(sap(S_P1, [[1, 2]]), sap(S_B + 1, [[1, 2]]), sap(S_D + 2, [[-2, 2]]), OP.mult)
    v.tensor_tensor(sap(S_P2, [[1, 2]]), sap(S_B + 2, [[-2, 2]]), sap(S_D + 1, [[1, 2]]), OP.mult)
    v.tensor_tensor(sap(S_P1 + 2, [[3, 2]]), sap(S_B, [[1, 2]]), sap(S_D + 1, [[-1, 2]]), OP.mult)
    # m = P2 - P1 = d x b  (orientation chosen so yy = +sqrt(q)*det)
    v.tensor_tensor(sap(S_M, [[1, 3]]), sap(S_P2, [[1, 3]]), sap(S_P1, [[1, 3]]), OP.subtract)
    # prods a*(b|d|m) -> r,p,det ; b*d -> s
    v.tensor_tensor(
        sap(S_PR, [[3, 3], [1, 3]]),
        sap(S_A, [[0, 3], [1, 3]]),
        sap(S_B, [[3, 3], [1, 3]]),
        OP.mult,
    )
    v.tensor_tensor(sap(S_PR + 9, [[1, 3]]), sap(S_B, [[1, 3]]), sap(S_D, [[1, 3]]), OP.mult)
    # segmented reduce -> dots (r,p,det,s,q) mini-planes. Unit inner stride on
    # both sides; strided-inner TT adds measured ~1.5 cyc/elem, so the 1x
    # reduce is just as fast and one instruction shorter.
    v.reduce_sum(
        map_(M_R, [[G, 5], [1, G]]),
        _ap(sa, S_PR, [[3, 5], [PER, G], [1, 3]]),
        axis=mybir.AxisListType.X,
    )
    # pq, rs
    v.tensor_tensor(
        map_(M_PQ, [[G, 2], [1, G]]),
        map_(M_P, [[-G, 2], [1, G]]),
        map_(M_Q, [[-G, 2], [1, G]]),
        OP.mult,
    )
    # xx -> full plane (kept for the tail's sign-of-xx correction)
    v.tensor_tensor(_ap(planes, toff, [[1, G]]), map_(M_PQ), map_(M_RS), OP.subtract)
    # rxx ~ 1/xx (51-ULP custom DVE op), sq = sqrt(q) on ScalarE
    v.reciprocal_approx_fast(map_(M_RXX), _ap(planes, toff, [[1, G]]))
    s.activation(map_(M_SQ), map_(M_Q), AF.Sqrt)
    # t2 = sq*det*rxx -> full plane
    v.tensor_tensor(map_(M_YY), map_(M_SQ), map_(M_DET), OP.mult)
    v.tensor_tensor(_ap(planes, Q + toff, [[1, G]]), map_(M_YY), map_(M_RXX), OP.mult)


def _emit_tail_scalar(nc, tsp, planes, toff, FD):
    """Chunk tail, ScalarE half: al2 = arctan(t2), s2 = sign(t2),
    cpl = -pi*[xx<0].  All funcs live in sigmoid_and_others (one table set).
    Returns the scratch tile for the deferred Vector half."""
    s = nc.scalar
    ts = tsp.tile([128, 3 * CHUNK_MAX], F32, tag="ts")
    ta = ts[:]

    def cap(k):
        return _ap(ta, k * CHUNK_MAX, [[1, FD]])

    s.activation(cap(0), _ap(planes, Q + toff, [[1, FD]]), AF.Arctan)   # al2
    s.activation(cap(1), _ap(planes, Q + toff, [[1, FD]]), AF.Sign)     # s2
    s.activation(cap(2), _ap(planes, toff, [[1, FD]]), AF.Sign, scale=-1.0)
    s.activation(cap(2), cap(2), AF.Copy, scale=-PI / 2, bias=-PI / 2)  # cpl
    return ts


def _emit_tail_vector(nc, ts, outp, y, toff, FD):
    """Chunk tail, Vector half: out = al2 + s2*cpl.  Emitted AFTER the next
    head's ops so the in-order DVE queue never stalls on the ScalarE chain."""
    v = nc.vector
    ot = outp.tile([128, CHUNK_MAX], F32, tag="o")
    ta = ts[:]

    def cap(k):
        return _ap(ta, k * CHUNK_MAX, [[1, FD]])

    v.tensor_tensor(cap(2), cap(1), cap(2), OP.mult)              # c = s2*cpl
    v.tensor_tensor(_ap(ot[:], 0, [[1, FD]]), cap(0), cap(2), OP.add)
    nc.gpsimd.dma_start(
        out=y.rearrange("(p q) -> p q", p=128)[:, toff : toff + FD],
        in_=_ap(ot[:], 0, [[1, FD]]),
    )


def build_kernel(atoms):
    c0, c1, c2, c3 = (3 * int(a) for a in atoms)
    nc = bacc.Bacc("TRN2", target_bir_lowering=False, debug=False)
    x = nc.dram_tensor("x", [ROWS_PER_CORE, 42], F32, kind="ExternalInput")
    y = nc.dram_tensor("y", [ROWS_PER_CORE], F32, kind="ExternalOutput")
    with TileContext(nc) as tc:
        with (
            tc.tile_pool(name="xp", bufs=2) as xp,
            tc.tile_pool(name="scp", bufs=1) as scp,
            tc.tile_pool(name="mp", bufs=1) as mp,
            tc.tile_pool(name="plp", bufs=1) as plp,
            tc.tile_pool(name="tsp", bufs=1) as tsp,
            tc.tile_pool(name="outp", bufs=2) as outp,
        ):
            pl_tile = plp.tile([128, 2 * Q], F32, tag="pl")
            planes = pl_tile[:]
            toff = 0
            chunk_start = 0
            pending = None
            for i, G in enumerate(TILES):
                _emit_head(nc, xp, scp, mp, x, planes, toff, G, c0, c1, c2, c3)
                if pending is not None:
                    _emit_tail_vector(nc, pending[0], outp, y, pending[1], pending[2])
                    pending = None
                toff += G
                if i in CHUNK_AFTER or i == len(TILES) - 1:
                    FD = toff - chunk_start
                    ts = _emit_tail_scalar(nc, tsp, planes, chunk_start, FD)
                    pending = (ts, chunk_start, FD)
                    chunk_start = toff
            if pending is not None:
                _emit_tail_vector(nc, pending[0], outp, y, pending[1], pending[2])
    nc.finalize()
    return nc


_CACHE = {}


def _get_nc(atoms):
    key = tuple(int(a) for a in atoms)
    if key not in _CACHE:
        _CACHE[key] = build_kernel(key)
    return _CACHE[key]


def run(x, atoms=(0, 4, 7, 11), **spmd_kwargs):
    """x: [B, 42] f32. Returns (y [B] f32, BassKernelResults)."""
    x = np.ascontiguousarray(np.asarray(x, dtype=np.float32))
    B = x.shape[0]
    total = N_CORES * ROWS_PER_CORE
    if B < total:
        # pad with replicated leading rows (valid, non-degenerate data)
        x = np.concatenate([x, x[: total - B]], axis=0)
    nc = _get_nc(atoms)
    shards = x.reshape(N_CORES, ROWS_PER_CORE, 42)
    in_maps = [{"x": shards[i]} for i in range(N_CORES)]
    res = run_bass_kernel_spmd(nc, in_maps, core_ids=list(range(N_CORES)), **spmd_kwargs)
    y = np.concatenate([r["y"] for r in res.results])[:B]
    return np.asarray(y, dtype=np.float32), res


def kernel(x, mask_matrix):
    mask = np.asarray(mask_matrix)
    atoms = tuple(int(i) for i in np.argmax(mask, axis=1))
    y, _ = run(x, atoms=atoms)
    return y


# revision 10
# speedup vs baseline: 1.2907x; 1.0442x over previous
"""Trainium2 Bass kernel for nn_DihedralAngleLayer.

Input:  x [2_000_000, 42] f32 (14 atoms x 3 coords per row),
        mask_matrix [4, 14] f32 one-hot carbon selector.
Output: dihedral angle per row, [2_000_000] f32.

Data-parallel across 8 NeuronCores: rows are padded to 8*250_112 and split
evenly. Each core owns rows in global partition-major order: partition p
handles rows [p*Q, (p+1)*Q), Q = rows/128.

Per tile (G rows of every partition) the Vector engine computes

    a = c0-c1, b = c2-c1, d = c3-c2, m = d x b  (3 split-AP multiplies, no
    duplicated subs), prods a*(b|d|m) and b*d, dots via two 5-plane strided
    adds (cheaper than the 1x-mode reduce), then
    xx = p*q - r*s, t2 = sqrt(q)*det / xx  (reciprocal_approx_fast).

The tail is a signed-ratio arctan: out = arctan(t2) + sign(t2)*(-pi*[xx<0]),
so per row only arctan + 2 tensor ops remain.  ScalarE ops are split so
heads touch only sqrt_and_others table funcs (Square/Sqrt) and tails only
sigmoid_and_others funcs (Arctan/Sign/Copy), bounding ACT table reloads to
two per chunk.  DVE model: (N+151)/0.96 ns per op -> ~46 elems/row + 15
instrs/tile; big G amortizes the per-instruction overhead.
"""

import numpy as np

import concourse.bacc as bacc
import concourse.bass as bass
import concourse.mybir as mybir
from concourse.bass_utils import run_bass_kernel_spmd
from concourse.tile import TileContext

AF = mybir.ActivationFunctionType
OP = mybir.AluOpType
F32 = mybir.dt.float32

PI = float(np.pi)

N_CORES = 8
# two small leading tiles start DVE sooner; small last chunk bounds end drain
TILES = [96, 128, 256, 256, 256, 256, 256, 256, 194]
CHUNK_AFTER = {3, 6}                 # tail chunks close after these + last
Q = sum(TILES)                       # rows per partition (1954)
ROWS_PER_CORE = 128 * Q              # 250_112
CHUNK_MAX = 768

# per-row scratch layout (period PER floats)
# a@0 b@3 d@6 m@9 n@12 P1@15 P2@18 P1n@21 P2n@24 prods@27 (det,xx,q x3)
PER = 36
S_A, S_B, S_D, S_M, S_N = 0, 3, 6, 9, 12
S_P1, S_P2, S_P1N, S_P2N, S_PR = 15, 18, 21, 24, 27
# mini planes ([G] each): det xx q rxx sq yy
M_DET, M_XX, M_Q, M_RXX, M_SQ, M_YY = range(6)


def _ap(base, off, dims):
    return bass.AP(
        base.tensor, base.offset + off, [list(base.ap[0])] + [list(d) for d in dims]
    )


def _emit_head(nc, xp, scp, mp, x, planes, toff, G, c0, c1, c2, c3):
    """Per-tile head: subs, cross, dots, xx/t2 -> full-length planes."""
    v, s = nc.vector, nc.scalar

    xt = xp.tile([128, G * 42], F32, tag="x")
    sc = scp.tile([128, G * PER], F32, tag="sc")
    mi = mp.tile([128, G * 6], F32, tag="mi")

    nc.sync.dma_start(
        out=xt[:],
        in_=x.rearrange("(p q) c -> p q c", p=128)[:, toff : toff + G, :],
    )

    xa, sa, ma = xt[:], sc[:], mi[:]

    def xap(off, dims):
        return _ap(xa, off, [[42, G]] + dims)

    def sap(off, dims=()):
        return _ap(sa, off, [[PER, G]] + list(dims))

    def map_(k, dims=None):
        return _ap(ma, k * G, dims if dims is not None else [[1, G]])

    # a = c0-c1 and b = c2-c1 fused (in0 strides over {c0,c2}, in1 reads c1 twice)
    v.tensor_tensor(
        sap(S_A, [[3, 2], [1, 3]]),
        xap(c0, [[c2 - c0, 2], [1, 3]]),
        xap(c1, [[0, 2], [1, 3]]),
        OP.subtract,
    )
    # d = c3-c2
    v.tensor_tensor(sap(S_D, [[1, 3]]), xap(c3, [[1, 3]]), xap(c2, [[1, 3]]), OP.subtract)
    # q prods = b*b on ScalarE (Square; sqrt_and_others set), overlaps DVE below
    s.activation(sap(S_PR + 6, [[1, 3]]), sap(S_B, [[1, 3]]), AF.Square)
    # Lagrange form: xx = (a x b).(d x b), det = a.(d x b) -- two cross
    # products, computed by three pairwise-merged split-AP multiplies.
    # m = d x b = P2-P1, n = a x b = P1n-P2n.
    # P1x: (P1[0],P1[1]) = (by,bz)*(dz,dx) ; (P1n[0],P1n[1]) = (ay,az)*(bz,bx)
    v.tensor_tensor(
        sap(S_P1, [[6, 2], [1, 2]]),
        sap(S_B + 1, [[-3, 2], [1, 2]]),
        sap(S_D + 2, [[-3, 2], [-2, 2]]),
        OP.mult,
    )
    # P2x: (P2[0],P2[1]) = (bz,bx)*(dy,dz) ; (P2n[0],P2n[1]) = (az,ax)*(by,bz)
    v.tensor_tensor(
        sap(S_P2, [[6, 2], [1, 2]]),
        sap(S_B + 2, [[-3, 2], [-2, 2]]),
        sap(S_D + 1, [[-3, 2], [1, 2]]),
        OP.mult,
    )
    # Pcx: (P1[2],P2[2]) = (bx,by)*(dy,dx) ; (P1n[2],P2n[2]) = (ax,ay)*(by,bx)
    v.tensor_tensor(
        sap(S_P1 + 2, [[6, 2], [3, 2]]),
        sap(S_B, [[-3, 2], [1, 2]]),
        sap(S_D + 1, [[-3, 2], [-1, 2]]),
        OP.mult,
    )
    # m = P2-P1, n = P1n-P2n in one op
    v.tensor_tensor(
        sap(S_M, [[3, 2], [1, 3]]),
        sap(S_P2, [[3, 2], [1, 3]]),
        sap(S_P1, [[9, 2], [1, 3]]),
        OP.subtract,
    )
    # prods (a*m -> det terms, n*m -> xx terms) in one op
    v.tensor_tensor(
        sap(S_PR, [[3, 2], [1, 3]]),
        sap(S_A, [[12, 2], [1, 3]]),
        sap(S_M, [[0, 2], [1, 3]]),
        OP.mult,
    )
    # segmented reduce -> (det, xx, q) mini-planes (unit inner stride)
    v.reduce_sum(
        map_(M_DET, [[G, 3], [1, G]]),
        _ap(sa, S_PR, [[3, 3], [PER, G], [1, 3]]),
        axis=mybir.AxisListType.X,
    )
    # xx -> full plane for the tail's sign-of-xx correction (ScalarE Copy,
    # present in every ACT table set)
    s.activation(_ap(planes, toff, [[1, G]]), map_(M_XX), AF.Copy)
    # rxx ~ 1/xx (51-ULP custom DVE op), sq = sqrt(q) on ScalarE
    v.reciprocal_approx_fast(map_(M_RXX), map_(M_XX))
    s.activation(map_(M_SQ), map_(M_Q), AF.Sqrt)
    # t2 = sq*det*rxx -> full plane
    v.tensor_tensor(map_(M_YY), map_(M_SQ), map_(M_DET), OP.mult)
    v.tensor_tensor(_ap(planes, Q + toff, [[1, G]]), map_(M_YY), map_(M_RXX), OP.mult)


def _emit_tail_scalar(nc, tsp, planes, toff, FD):
    """Chunk tail, ScalarE half: al2 = arctan(t2), s2 = sign(t2),
    cpl = -pi*[xx<0].  All funcs live in sigmoid_and_others (one table set).
    Returns the scratch tile for the deferred Vector half."""
    s = nc.scalar
    ts = tsp.tile([128, 3 * CHUNK_MAX], F32, tag="ts")
    ta = ts[:]

    def cap(k):
        return _ap(ta, k * CHUNK_MAX, [[1, FD]])

    s.activation(cap(0), _ap(planes, Q + toff, [[1, FD]]), AF.Arctan)   # al2
    s.activation(cap(1), _ap(planes, Q + toff, [[1, FD]]), AF.Sign)     # s2
    s.activation(cap(2), _ap(planes, toff, [[1, FD]]), AF.Sign, scale=-1.0)
    s.activation(cap(2), cap(2), AF.Copy, scale=-PI / 2, bias=-PI / 2)  # cpl
    return ts


def _emit_tail_vector(nc, ts, outp, y, toff, FD):
    """Chunk tail, Vector half: out = al2 + s2*cpl.  Emitted AFTER the next
    head's ops so the in-order DVE queue never stalls on the ScalarE chain."""
    v = nc.vector
    ot = outp.tile([128, CHUNK_MAX], F32, tag="o")
    ta = ts[:]

    def cap(k):
        return _ap(ta, k * CHUNK_MAX, [[1, FD]])

    v.tensor_tensor(cap(2), cap(1), cap(2), OP.mult)              # c = s2*cpl
    v.tensor_tensor(_ap(ot[:], 0, [[1, FD]]), cap(0), cap(2), OP.add)
    nc.gpsimd.dma_start(
        out=y.rearrange("(p q) -> p q", p=128)[:, toff : toff + FD],
        in_=_ap(ot[:], 0, [[1, FD]]),
    )


def build_kernel(atoms):
    c0, c1, c2, c3 = (3 * int(a) for a in atoms)
    nc = bacc.Bacc("TRN2", target_bir_lowering=False, debug=False)
    x = nc.dram_tensor("x", [ROWS_PER_CORE, 42], F32, kind="ExternalInput")
    y = nc.dram_tensor("y", [ROWS_PER_CORE], F32, kind="ExternalOutput")
    with TileContext(nc) as tc:
        with (
            tc.tile_pool(name="xp", bufs=3) as xp,
            tc.tile_pool(name="scp", bufs=1) as scp,
            tc.tile_pool(name="mp", bufs=1) as mp,
            tc.tile_pool(name="plp", bufs=1) as plp,
            tc.tile_pool(name="tsp", bufs=1) as tsp,
            tc.tile_pool(name="outp", bufs=2) as outp,
        ):
            pl_tile = plp.tile([128, 2 * Q], F32, tag="pl")
            planes = pl_tile[:]
            toff = 0
            chunk_start = 0
            pending = None
            for i, G in enumerate(TILES):
                _emit_head(nc, xp, scp, mp, x, planes, toff, G, c0, c1, c2, c3)
                if pending is not None:
                    _emit_tail_vector(nc, pending[0], outp, y, pending[1], pending[2])
                    pending = None
                toff += G
                if i in CHUNK_AFTER or i == len(TILES) - 1:
                    FD = toff - chunk_start
                    ts = _emit_tail_scalar(nc, tsp, planes, chunk_start, FD)
                    pending = (ts, chunk_start, FD)
                    chunk_start = toff
            if pending is not None:
                _emit_tail_vector(nc, pending[0], outp, y, pending[1], pending[2])
    nc.finalize()
    return nc


_CACHE = {}


def _get_nc(atoms):
    key = tuple(int(a) for a in atoms)
    if key not in _CACHE:
        _CACHE[key] = build_kernel(key)
    return _CACHE[key]


def run(x, atoms=(0, 4, 7, 11), **spmd_kwargs):
    """x: [B, 42] f32. Returns (y [B] f32, BassKernelResults)."""
    x = np.ascontiguousarray(np.asarray(x, dtype=np.float32))
    B = x.shape[0]
    total = N_CORES * ROWS_PER_CORE
    if B < total:
        # pad with replicated leading rows (valid, non-degenerate data)
        x = np.concatenate([x, x[: total - B]], axis=0)
    nc = _get_nc(atoms)
    shards = x.reshape(N_CORES, ROWS_PER_CORE, 42)
    in_maps = [{"x": shards[i]} for i in range(N_CORES)]
    res = run_bass_kernel_spmd(nc, in_maps, core_ids=list(range(N_CORES)), **spmd_kwargs)
    y = np.concatenate([r["y"] for r in res.results])[:B]
    return np.asarray(y, dtype=np.float32), res


def kernel(x, mask_matrix):
    mask = np.asarray(mask_matrix)
    atoms = tuple(int(i) for i in np.argmax(mask, axis=1))
    y, _ = run(x, atoms=atoms)
    return y


# revision 12
# speedup vs baseline: 1.3440x; 1.0413x over previous
"""Trainium2 Bass kernel for nn_DihedralAngleLayer.

Input:  x [2_000_000, 42] f32 (14 atoms x 3 coords per row),
        mask_matrix [4, 14] f32 one-hot carbon selector.
Output: dihedral angle per row, [2_000_000] f32.

Data-parallel across 8 NeuronCores: rows are padded to 8*250_112 and split
evenly. Each core owns rows in global partition-major order: partition p
handles rows [p*Q, (p+1)*Q), Q = rows/128.

Per tile (G rows of every partition) the Vector engine computes

    a = c0-c1, b = c2-c1, d = c3-c2, m = d x b  (3 split-AP multiplies, no
    duplicated subs), prods a*(b|d|m) and b*d, dots via two 5-plane strided
    adds (cheaper than the 1x-mode reduce), then
    xx = p*q - r*s, t2 = sqrt(q)*det / xx  (reciprocal_approx_fast).

The tail is a signed-ratio arctan: out = arctan(t2) + sign(t2)*(-pi*[xx<0]),
so per row only arctan + 2 tensor ops remain.  ScalarE ops are split so
heads touch only sqrt_and_others table funcs (Square/Sqrt) and tails only
sigmoid_and_others funcs (Arctan/Sign/Copy), bounding ACT table reloads to
two per chunk.  DVE model: (N+151)/0.96 ns per op -> ~46 elems/row + 15
instrs/tile; big G amortizes the per-instruction overhead.
"""

import numpy as np

import concourse.bacc as bacc
import concourse.bass as bass
import concourse.mybir as mybir
from concourse.bass_utils import run_bass_kernel_spmd
from concourse.tile import TileContext

AF = mybir.ActivationFunctionType
OP = mybir.AluOpType
F32 = mybir.dt.float32

PI = float(np.pi)

N_CORES = 8
# geometric ramp: DVE starts on the first small tile and never starves while
# the (faster) DMA stream builds its lead; small last chunk bounds end drain
TILES = [64, 96, 128, 192, 256, 256, 256, 256, 256, 194]
CHUNK_AFTER = {4, 7, 8}              # tail chunks close after these + last
Q = sum(TILES)                       # rows per partition (1954)
ROWS_PER_CORE = 128 * Q              # 250_112
CHUNK_MAX = 768

# per-row scratch layout (period PER floats)
# a@0 b@3 d@6 m@9 n@12 P1@15 P2@18 P1n@21 P2n@24 prods@27 (det,xx,q x3)
PER = 36
S_A, S_B, S_D, S_M, S_N = 0, 3, 6, 9, 12
S_P1, S_P2, S_P1N, S_P2N, S_PR = 15, 18, 21, 24, 27
# mini planes ([G] each): det xx q rxx sq yy
M_DET, M_XX, M_Q, M_RXX, M_SQ, M_YY = range(6)


def _ap(base, off, dims):
    return bass.AP(
        base.tensor, base.offset + off, [list(base.ap[0])] + [list(d) for d in dims]
    )


def _emit_head(nc, xp, scp, mp, x, planes, toff, G, c0, c1, c2, c3):
    """Per-tile head: subs, cross, dots, xx/t2 -> full-length planes."""
    v, s = nc.vector, nc.scalar

    xt = xp.tile([128, G * 42], F32, tag="x")
    sc = scp.tile([128, G * PER], F32, tag="sc")
    mi = mp.tile([128, G * 6], F32, tag="mi")

    nc.sync.dma_start(
        out=xt[:],
        in_=x.rearrange("(p q) c -> p q c", p=128)[:, toff : toff + G, :],
    )

    xa, sa, ma = xt[:], sc[:], mi[:]

    def xap(off, dims):
        return _ap(xa, off, [[42, G]] + dims)

    def sap(off, dims=()):
        return _ap(sa, off, [[PER, G]] + list(dims))

    def map_(k, dims=None):
        return _ap(ma, k * G, dims if dims is not None else [[1, G]])

    # a = c0-c1 and b = c2-c1 fused (in0 strides over {c0,c2}, in1 reads c1 twice)
    v.tensor_tensor(
        sap(S_A, [[3, 2], [1, 3]]),
        xap(c0, [[c2 - c0, 2], [1, 3]]),
        xap(c1, [[0, 2], [1, 3]]),
        OP.subtract,
    )
    # d = c3-c2
    v.tensor_tensor(sap(S_D, [[1, 3]]), xap(c3, [[1, 3]]), xap(c2, [[1, 3]]), OP.subtract)
    # q prods = b*b on ScalarE (Square; sqrt_and_others set), overlaps DVE below
    s.activation(sap(S_PR + 6, [[1, 3]]), sap(S_B, [[1, 3]]), AF.Square)
    # Lagrange form: xx = (a x b).(d x b), det = a.(d x b) -- two cross
    # products, computed by three pairwise-merged split-AP multiplies.
    # m = d x b = P2-P1, n = a x b = P1n-P2n.
    # P1x: (P1[0],P1[1]) = (by,bz)*(dz,dx) ; (P1n[0],P1n[1]) = (ay,az)*(bz,bx)
    v.tensor_tensor(
        sap(S_P1, [[6, 2], [1, 2]]),
        sap(S_B + 1, [[-3, 2], [1, 2]]),
        sap(S_D + 2, [[-3, 2], [-2, 2]]),
        OP.mult,
    )
    # P2x: (P2[0],P2[1]) = (bz,bx)*(dy,dz) ; (P2n[0],P2n[1]) = (az,ax)*(by,bz)
    v.tensor_tensor(
        sap(S_P2, [[6, 2], [1, 2]]),
        sap(S_B + 2, [[-3, 2], [-2, 2]]),
        sap(S_D + 1, [[-3, 2], [1, 2]]),
        OP.mult,
    )
    # Pcx: (P1[2],P2[2]) = (bx,by)*(dy,dx) ; (P1n[2],P2n[2]) = (ax,ay)*(by,bx)
    v.tensor_tensor(
        sap(S_P1 + 2, [[6, 2], [3, 2]]),
        sap(S_B, [[-3, 2], [1, 2]]),
        sap(S_D + 1, [[-3, 2], [-1, 2]]),
        OP.mult,
    )
    # m = P2-P1, n = P1n-P2n in one op
    v.tensor_tensor(
        sap(S_M, [[3, 2], [1, 3]]),
        sap(S_P2, [[3, 2], [1, 3]]),
        sap(S_P1, [[9, 2], [1, 3]]),
        OP.subtract,
    )
    # prods (a*m -> det terms, n*m -> xx terms) in one op
    v.tensor_tensor(
        sap(S_PR, [[3, 2], [1, 3]]),
        sap(S_A, [[12, 2], [1, 3]]),
        sap(S_M, [[0, 2], [1, 3]]),
        OP.mult,
    )
    # segmented reduce -> (det, xx, q) mini-planes (unit inner stride)
    v.reduce_sum(
        map_(M_DET, [[G, 3], [1, G]]),
        _ap(sa, S_PR, [[3, 3], [PER, G], [1, 3]]),
        axis=mybir.AxisListType.X,
    )
    # xx -> full plane for the tail's sign-of-xx correction (ScalarE Copy,
    # present in every ACT table set)
    s.activation(_ap(planes, toff, [[1, G]]), map_(M_XX), AF.Copy)
    # rxx ~ 1/xx (51-ULP custom DVE op), sq = sqrt(q) on ScalarE
    v.reciprocal_approx_fast(map_(M_RXX), map_(M_XX))
    s.activation(map_(M_SQ), map_(M_Q), AF.Sqrt)
    # t2 = sq*det*rxx -> full plane
    v.tensor_tensor(map_(M_YY), map_(M_SQ), map_(M_DET), OP.mult)
    v.tensor_tensor(_ap(planes, Q + toff, [[1, G]]), map_(M_YY), map_(M_RXX), OP.mult)


def _emit_tail_scalar(nc, tsp, planes, toff, FD):
    """Chunk tail, ScalarE half: al2 = arctan(t2), s2 = sign(t2),
    cpl = -pi*[xx<0].  All funcs live in sigmoid_and_others (one table set).
    Returns the scratch tile for the deferred Vector half."""
    s = nc.scalar
    ts = tsp.tile([128, 3 * CHUNK_MAX], F32, tag="ts")
    ta = ts[:]

    def cap(k):
        return _ap(ta, k * CHUNK_MAX, [[1, FD]])

    s.activation(cap(0), _ap(planes, Q + toff, [[1, FD]]), AF.Arctan)   # al2
    s.activation(cap(1), _ap(planes, Q + toff, [[1, FD]]), AF.Sign)     # s2
    s.activation(cap(2), _ap(planes, toff, [[1, FD]]), AF.Sign, scale=-1.0)
    s.activation(cap(2), cap(2), AF.Copy, scale=-PI / 2, bias=-PI / 2)  # cpl
    return ts


def _emit_tail_vector(nc, ts, outp, y, toff, FD):
    """Chunk tail, Vector half: out = al2 + s2*cpl.  Emitted AFTER the next
    head's ops so the in-order DVE queue never stalls on the ScalarE chain."""
    v = nc.vector
    ot = outp.tile([128, CHUNK_MAX], F32, tag="o")
    ta = ts[:]

    def cap(k):
        return _ap(ta, k * CHUNK_MAX, [[1, FD]])

    v.tensor_tensor(cap(2), cap(1), cap(2), OP.mult)              # c = s2*cpl
    v.tensor_tensor(_ap(ot[:], 0, [[1, FD]]), cap(0), cap(2), OP.add)
    nc.sync.dma_start(
        out=y.rearrange("(p q) -> p q", p=128)[:, toff : toff + FD],
        in_=_ap(ot[:], 0, [[1, FD]]),
    )


def build_kernel(atoms):
    c0, c1, c2, c3 = (3 * int(a) for a in atoms)
    nc = bacc.Bacc("TRN2", target_bir_lowering=False, debug=False)
    x = nc.dram_tensor("x", [ROWS_PER_CORE, 42], F32, kind="ExternalInput")
    y = nc.dram_tensor("y", [ROWS_PER_CORE], F32, kind="ExternalOutput")
    with TileContext(nc) as tc:
        with (
            tc.tile_pool(name="xp", bufs=3) as xp,
            tc.tile_pool(name="scp", bufs=1) as scp,
            tc.tile_pool(name="mp", bufs=1) as mp,
            tc.tile_pool(name="plp", bufs=1) as plp,
            tc.tile_pool(name="tsp", bufs=1) as tsp,
            tc.tile_pool(name="outp", bufs=2) as outp,
        ):
            pl_tile = plp.tile([128, 2 * Q], F32, tag="pl")
            planes = pl_tile[:]
            toff = 0
            chunk_start = 0
            pending = None
            for i, G in enumerate(TILES):
                _emit_head(nc, xp, scp, mp, x, planes, toff, G, c0, c1, c2, c3)
                if pending is not None:
                    _emit_tail_vector(nc, pending[0], outp, y, pending[1], pending[2])
                    pending = None
                toff += G
                if i in CHUNK_AFTER or i == len(TILES) - 1:
                    FD = toff - chunk_start
                    ts = _emit_tail_scalar(nc, tsp, planes, chunk_start, FD)
                    pending = (ts, chunk_start, FD)
                    chunk_start = toff
            if pending is not None:
                _emit_tail_vector(nc, pending[0], outp, y, pending[1], pending[2])
    nc.finalize()
    return nc


_CACHE = {}


def _get_nc(atoms):
    key = tuple(int(a) for a in atoms)
    if key not in _CACHE:
        _CACHE[key] = build_kernel(key)
    return _CACHE[key]


def run(x, atoms=(0, 4, 7, 11), **spmd_kwargs):
    """x: [B, 42] f32. Returns (y [B] f32, BassKernelResults)."""
    x = np.ascontiguousarray(np.asarray(x, dtype=np.float32))
    B = x.shape[0]
    total = N_CORES * ROWS_PER_CORE
    if B < total:
        # pad with replicated leading rows (valid, non-degenerate data)
        x = np.concatenate([x, x[: total - B]], axis=0)
    nc = _get_nc(atoms)
    shards = x.reshape(N_CORES, ROWS_PER_CORE, 42)
    in_maps = [{"x": shards[i]} for i in range(N_CORES)]
    res = run_bass_kernel_spmd(nc, in_maps, core_ids=list(range(N_CORES)), **spmd_kwargs)
    y = np.concatenate([r["y"] for r in res.results])[:B]
    return np.asarray(y, dtype=np.float32), res


def kernel(x, mask_matrix):
    mask = np.asarray(mask_matrix)
    atoms = tuple(int(i) for i in np.argmax(mask, axis=1))
    y, _ = run(x, atoms=atoms)
    return y
